# revision 60
# baseline (speedup 1.0000x reference)
"""GTrXL layer (TransformerXL attention + GRU gating) on 8 TRN2 NeuronCores.

Sharding: pure data-parallel over batch (BS=8 -> 1 batch element per core).
No collectives. Per-core Bass/Tile kernel computes the full layer for its
batch element.

Layout convention on-chip: activations are kept TRANSPOSED [feature, token]
(feature on partitions, 128-chunks).

Precision strategy: all dense matmuls run in fp8-e4m3 with DoubleRow perf
mode (2 contraction k-tiles per pass, full 128-wide stationary -> psum
[128, 256] per 128-cycle pass). Weights are scaled by 256 on the host before
fp8 quantization (keeps values out of the subnormal range); every PSUM
consumer applies a 2^-8 scale (alternating DVE tensor_scalar / ACT
activation(Identity) — only those engines may touch PSUM; GPSIMD cannot).
Activations quantize to fp8 at natural scale. Elementwise math (LN, GRU
combine) stays f32; logits/es stay bf16.

LN1 runs feature-major on a host-transposed input: mean/E[x^2] via
ones-matmul partition reductions (bf16), then normalize + fp8 quantize.

Relative-shift: per head, pos scores P[i, rel] for the needed rel range
[384-128*ic, 1024) of all 4 query chunks are written in one DMA to a 4-row
DRAM scratch (row stride 1536, fp8, tail 512 cols pre-filled with -240).
The shifted read  shifted[i, j] = P[i, 511 - 128*ic + j - i]  is a strided
DMA (row step 4*1536-1, sub-row offset 511-128*ic — the correct global TrXL
shift; the staged baseline used the local chunk index here, which is subtly
wrong but heavily damped by the GRU gates). The -240 pad lands exactly on
the masked region and exp()s to ~0.

Attention is a depth-2 software pipeline over heads: pos+shift(h) |
content+exp(h-1) | transpose+AV(h-2). The shifted pos scores are added to
the content logits IN PSUM by an identity-matmul accumulate (PE), exp reads
PSUM directly and emits softmax denominators via accum_out; normalization is
folded into the es tile on GPSIMD (SBUF-only) with a 128x fp8 scale, and the
AV output is rescaled by 2^-7.
"""

import sys

if '/opt/trn_rl_repo' not in sys.path:
    sys.path.insert(0, '/opt/trn_rl_repo')

import numpy as np
import ml_dtypes

import concourse.bass as bass
import concourse.tile as tile
from concourse import bacc, mybir
from concourse.bass_utils import run_bass_kernel_spmd
from concourse.masks import make_identity

BF16 = mybir.dt.bfloat16
F32 = mybir.dt.float32
FP8 = mybir.dt.float8e4

HEAD_NUM, HEAD_DIM = 16, 64
D, HID = 1024, 4096
CUR, PREV, BS = 512, 512, 8
FULL = CUR + PREV
EPS = 1e-5
SCALE = 1.0 / (HEAD_DIM ** 0.5)
P = 128
DC = D // P          # 8 feature chunks
HC = HID // P        # 32 hidden chunks
TCF = FULL // P      # 8 full-token chunks
TCC = CUR // P       # 4 query-token chunks
NEG = -1.0e30
WS = 256.0           # host-side weight scale before fp8 quantization
ISW = 1.0 / WS       # psum de-scale
ES_S = 128.0         # softmax-weight fp8 scale
IES = 1.0 / ES_S

AluOp = mybir.AluOpType
Act = mybir.ActivationFunctionType
DR = mybir.MatmulPerfMode.DoubleRow


def _dram_in(dram, name, shape, dtype):
    return dram.tile(list(shape), dtype, kind="ExternalInput", name=name,
                     uniquify=False)


def _dr_quads(nc, ps, segs):
    """Fill psum [128, 512] via DoubleRow chains (full 128-wide stationary).

    segs: list of (w, x, c0, t0, kpairs) — accumulate over all segs:
      ps[n, t] += sum_k w[k, c0+n] * x[k, t0+t]   (k over kpairs*256 lanes)
    w, x are [P, 2*kpairs.., *] fp8 tiles (chunk dim second).
    """
    for qh in range(2):
        out = ps[:, qh * 256:qh * 256 + 256]
        total = sum(len(s[4]) for s in segs)
        i = 0
        for (w, x, c0, t0, kpairs) in segs:
            for m in kpairs:
                nc.tensor.matmul(
                    out,
                    lhsT=w[:, 2 * m:2 * m + 2, c0:c0 + P],
                    rhs=x[:, 2 * m:2 * m + 2,
                          t0 + qh * 256:t0 + qh * 256 + 256],
                    perf_mode=DR,
                    start=(i == 0), stop=(i == total - 1))
                i += 1


def _build():
    nc = bacc.Bacc("TRN2", target_bir_lowering=False)
    with tile.TileContext(nc) as tc:
        _emit(nc, tc)
    nc.compile()
    return nc


def _emit(nc, tc):
    from contextlib import ExitStack

    with ExitStack() as root:
        dram = root.enter_context(tc.tile_pool(name="io", bufs=1, space="DRAM"))

        # ---------------- DRAM I/O ----------------
        x_full = _dram_in(dram, "x_full", (D, FULL), F32)
        inpT_d = _dram_in(dram, "inpT", (D, CUR), F32)
        posT_d = _dram_in(dram, "posT8", (D, FULL), FP8)
        u_d = _dram_in(dram, "u_t", (P, DC), F32)
        v_d = _dram_in(dram, "v_t", (P, DC), F32)
        ln1g_d = _dram_in(dram, "ln1_g_t", (P, DC), F32)
        ln1b_d = _dram_in(dram, "ln1_b_t", (P, DC), F32)
        ln2g_d = _dram_in(dram, "ln2_g_t", (P, DC), F32)
        ln2b_d = _dram_in(dram, "ln2_b_t", (P, DC), F32)
        bkvK_d = _dram_in(dram, "bkvK_t", (P, DC), F32)
        bkvV_d = _dram_in(dram, "bkvV_row", (1, D), F32)
        bq_d = _dram_in(dram, "bq_t", (P, DC), F32)
        bpos_d = _dram_in(dram, "bpos_t", (P, DC), F32)
        bproj_d = _dram_in(dram, "bproj_t", (P, DC), F32)
        b1_d = _dram_in(dram, "b1_t", (P, HC), F32)
        b2_d = _dram_in(dram, "b2_t", (P, DC), F32)
        nbg1_d = _dram_in(dram, "nbg1_t", (P, DC), F32)
        nbg2_d = _dram_in(dram, "nbg2_t", (P, DC), F32)

        wkvK_d = _dram_in(dram, "WkvK8", (D, D), FP8)
        wkvV_d = _dram_in(dram, "WkvV8", (D, D), FP8)
        wq_d = _dram_in(dram, "Wq8", (D, D), FP8)
        wpos_d = _dram_in(dram, "Wpos8", (D, D), FP8)
        wproj_d = _dram_in(dram, "Wproj8", (D, D), FP8)
        gw_d = {}
        for g in (1, 2):
            for m in ("Wr", "Ur", "Wz", "Uz", "Wg", "Ug"):
                gw_d[(g, m)] = _dram_in(dram, f"g{g}_{m}8", (D, D), FP8)
        w1_d = _dram_in(dram, "mlp_W18", (D, HID), FP8)
        w2_d = _dram_in(dram, "mlp_W28", (HID, D), FP8)

        # transposed output [D, CUR]; host transposes back
        out_d = dram.tile([D, CUR], F32, kind="ExternalOutput", name="out",
                          uniquify=False)

        # per-head scratch: 4 sub-rows (one per query chunk) of width 1536,
        # fp8, tail 512 cols pre-filled with -240 (acts as -inf after exp)
        n_scr = 4
        scr = [dram.tile([P, TCC, 1536], FP8, name=f"scr{s}")
               for s in range(n_scr)]

        # ---------------- constants ----------------
        const = root.enter_context(tc.tile_pool(name="const", bufs=1))
        ident_f = const.tile([P, P], F32)
        make_identity(nc, ident_f)
        ident_b = const.tile([P, P], BF16)
        make_identity(nc, ident_b)
        ident_8 = const.tile([P, P], FP8)
        make_identity(nc, ident_8)
        ones_red8 = const.tile([P, 1], FP8)
        nc.vector.memset(ones_red8, 1.0)
        ones_b = const.tile([P, 1], BF16)
        nc.vector.memset(ones_b, 1.0)
        eps_t = const.tile([P, 1], F32)
        nc.vector.memset(eps_t, EPS)

        def cload(name, dref, shape, dtype=F32):
            t = const.tile(list(shape), dtype, name=name)
            nc.sync.dma_start(out=t, in_=dref[:])
            return t

        u_sb = cload("u_sb", u_d, (P, DC))
        v_sb = cload("v_sb", v_d, (P, DC))
        ln2g_sb = cload("ln2g_sb", ln2g_d, (P, DC))
        ln2b_sb = cload("ln2b_sb", ln2b_d, (P, DC))
        bkvK_sb = cload("bkvK_sb", bkvK_d, (P, DC))
        bq_sb = cload("bq_sb", bq_d, (P, DC))
        bpos_sb = cload("bpos_sb", bpos_d, (P, DC))
        bproj_sb = cload("bproj_sb", bproj_d, (P, DC))
        b1_sb = cload("b1_sb", b1_d, (P, HC))
        b2_sb = cload("b2_sb", b2_d, (P, DC))
        nbg1_sb = cload("nbg1_sb", nbg1_d, (P, DC))
        nbg2_sb = cload("nbg2_sb", nbg2_d, (P, DC))
        ln1g_sb = cload("ln1g_sb", ln1g_d, (P, DC))
        ln1b_sb = cload("ln1b_sb", ln1b_d, (P, DC))
        # V bias broadcast to all partitions (free-dim varying)
        bvV_sb = const.tile([P, D], F32, name="bvV_sb")
        nc.sync.dma_start(out=bvV_sb, in_=bass.AP(
            tensor=bkvV_d.tensor, offset=bkvV_d.offset, ap=[[0, P], [1, D]]))

        padw = const.tile([P, TCC, 512], FP8)
        nc.vector.memset(padw, -240.0)
        for s in range(n_scr):
            nc.sync.dma_start(out=scr[s][:, :, 1024:1536], in_=padw)

        # engine rotation for copies / elementwise
        vecs = [nc.vector, nc.gpsimd]

        def VE(i):
            return vecs[i % 2]

        def psum_sb(i, out, ps, bias_ap):
            """out = ps * ISW + bias  (PSUM-legal engines only)."""
            if i % 2 == 0:
                nc.vector.tensor_scalar(out=out, in0=ps, scalar1=ISW,
                                        scalar2=bias_ap, op0=AluOp.mult,
                                        op1=AluOp.add)
            else:
                nc.scalar.activation(out=out, in_=ps, func=Act.Identity,
                                     scale=ISW, bias=bias_ap)

        # phase-scoped psum pools (PSUM is only 8 banks; attention needs them)
        psum_box = {}

        def PS():
            return psum_box["p"].tile([P, 512], F32, name="ps", tag="ps")

        def SM():
            return psum_box["s"].tile([1, 512], F32, name="sm", tag="sm")

        def mk(name, shape, dtype, side):
            t, fr = tc.tile(list(shape), dtype, name=name, side=side)
            return t, fr

        x1T8, fr_x1T = mk("x1T8", (P, DC, FULL), FP8, "left")

        # ============ Phase 1+2 fused: rT first (warms PE), then LN1 with
        # ============ V/kT/q matmuls interleaved as token chunks complete
        kT, fr_kT = mk("kT", (P, DC, FULL), FP8, "right")
        v_nat, fr_v = mk("v_nat", (P, TCF, D), FP8, "right")
        rT, fr_rT = mk("rT", (P, DC, FULL), FP8, "right")
        quT, fr_quT = mk("quT", (P, DC, CUR), FP8, "right")
        qvT, fr_qvT = mk("qvT", (P, DC, CUR), FP8, "right")

        KP = [0, 1, 2, 3]  # the 4 k-chunk pairs covering D=1024

        with ExitStack() as ph:
            psum_box["p"] = ph.enter_context(
                tc.tile_pool(name="psum12", bufs=4, space="PSUM"))
            wkvp = ph.enter_context(tc.tile_pool(name="wkvp", bufs=1, side="right"))
            wpos = wkvp.tile([P, DC, D], FP8)
            nc.scalar.dma_start(out=wpos, in_=wpos_d[:].rearrange("(kc p) n -> p kc n", p=P))
            posT_sb = wkvp.tile([P, DC, FULL], FP8)
            nc.scalar.dma_start(out=posT_sb, in_=posT_d[:].rearrange("(kc p) f -> p kc f", p=P))
            wkvK = wkvp.tile([P, DC, D], FP8)
            nc.scalar.dma_start(out=wkvK, in_=wkvK_d[:].rearrange("(kc p) n -> p kc n", p=P))
            wkvV = wkvp.tile([P, DC, D], FP8)
            nc.scalar.dma_start(out=wkvV, in_=wkvV_d[:].rearrange("(kc p) n -> p kc n", p=P))
            wq = wkvp.tile([P, DC, D], FP8)
            nc.scalar.dma_start(out=wq, in_=wq_d[:].rearrange("(kc p) n -> p kc n", p=P))

            # rT only needs pos inputs — keeps the PE busy during LN1
            for n in range(DC):
                for fh in range(2):
                    ps = PS()
                    _dr_quads(nc, ps, [(wpos, posT_sb, n * P, fh * 512, KP)])
                    psum_sb(n + fh, rT[:, n, fh * 512:(fh + 1) * 512],
                            ps, bpos_sb[:, n:n + 1])

            # ---- feature-major LN1: x arrives pre-transposed [D, FULL] ----
            lnw = ph.enter_context(tc.tile_pool(name="lnw", bufs=1, side="left"))
            xT_f = lnw.tile([P, DC, FULL], F32, name="xT_f")
            xr = x_full[:].rearrange("(kc p) t -> p kc t", p=P)
            for k in range(DC):
                nc.sync.dma_start(out=xT_f[:, k, :], in_=xr[:, k, :])
            xb = lnw.tile([P, DC, FULL], BF16, name="xb")
            sq_b = lnw.tile([P, DC, FULL], BF16, name="sq_b")
            rowp = ph.enter_context(tc.tile_pool(name="rowp", bufs=4, space="PSUM"))
            s_mean = [rowp.tile([1, 512], F32, name="s_mean", tag="row")
                      for _ in range(2)]
            s_sq = [rowp.tile([1, 512], F32, name="s_sq", tag="row")
                    for _ in range(2)]
            for k in range(DC):
                nc.gpsimd.tensor_copy(xb[:, k, :], xT_f[:, k, :])
                VE(k).tensor_mul(sq_b[:, k, :], xb[:, k, :], xb[:, k, :])
            for fh in range(2):
                for k in range(DC):
                    nc.tensor.matmul(
                        s_mean[fh], lhsT=ones_b,
                        rhs=xb[:, k, fh * 512:(fh + 1) * 512],
                        start=(k == 0), stop=(k == DC - 1))
                for k in range(DC):
                    nc.tensor.matmul(
                        s_sq[fh], lhsT=ones_b,
                        rhs=sq_b[:, k, fh * 512:(fh + 1) * 512],
                        start=(k == 0), stop=(k == DC - 1))
            mrow = lnw.tile([1, FULL], F32, name="mrow")
            vrow = lnw.tile([1, FULL], F32, name="vrow")
            for fh in range(2):
                sl = slice(fh * 512, (fh + 1) * 512)
                nc.vector.tensor_scalar_mul(mrow[:, sl], s_mean[fh], 1.0 / D)
                nc.vector.tensor_scalar_mul(vrow[:, sl], s_sq[fh], 1.0 / D)
            # var = E[x^2] - mean^2 ; rstd = 1/sqrt(var+eps)
            m2row = lnw.tile([1, FULL], F32, name="m2row")
            nc.vector.tensor_mul(m2row, mrow, mrow)
            nc.vector.tensor_sub(vrow, vrow, m2row)
            srow = lnw.tile([1, FULL], F32, name="srow")
            nc.scalar.activation(out=srow, in_=vrow, func=Act.Sqrt,
                                 bias=eps_t[0:1, :])
            rrow = lnw.tile([1, FULL], F32, name="rrow")
            nc.vector.reciprocal(out=rrow, in_=srow)
            meanB = lnw.tile([P, FULL], F32, name="meanB")
            nc.gpsimd.partition_broadcast(meanB, mrow)
            rstdB = lnw.tile([P, FULL], F32, name="rstdB")
            nc.gpsimd.partition_broadcast(rstdB, rrow)
            tw = ph.enter_context(tc.tile_pool(name="tw", bufs=3, side="left"))
            for k in range(DC):
                t1 = tw.tile([P, FULL], F32, name="t1")
                VE(k).tensor_sub(t1, xT_f[:, k, :], meanB)
                VE(k).tensor_mul(t1, t1, rstdB)
                VE(k + 1).tensor_scalar(out=x1T8[:, k, :], in0=t1,
                                        scalar1=ln1g_sb[:, k:k + 1],
                                        scalar2=ln1b_sb[:, k:k + 1],
                                        op0=AluOp.mult, op1=AluOp.add)

            # ---- kv / q projections ----
            qw = ph.enter_context(tc.tile_pool(name="qw", bufs=3, side="left"))
            for n in range(DC):
                ps = PS()
                _dr_quads(nc, ps, [(wq, x1T8, n * P, CUR, KP)])
                qn = qw.tile([P, 512], F32, name="qn")
                nc.scalar.activation(out=qn, in_=ps, func=Act.Identity,
                                     scale=ISW, bias=bq_sb[:, n:n + 1])
                nc.scalar.activation(out=quT[:, n, :], in_=qn,
                                     func=Act.Identity, scale=1.0,
                                     bias=u_sb[:, n:n + 1])
                nc.gpsimd.tensor_scalar_add(qvT[:, n, :], qn,
                                            v_sb[:, n:n + 1])
            for th in range(2):
                for n in range(DC):
                    ps = PS()
                    _dr_quads(nc, ps, [(wkvK, x1T8, n * P, th * 512, KP)])
                    psum_sb(n + th, kT[:, n, th * 512:(th + 1) * 512],
                            ps, bkvK_sb[:, n:n + 1])
            for tcx in range(TCF):
                for nh in range(2):
                    ps = PS()
                    _dr_quads(nc, ps, [(x1T8, wkvV, tcx * P, nh * 512, KP)])
                    nc.vector.scalar_tensor_tensor(
                        out=v_nat[:, tcx, nh * 512:(nh + 1) * 512], in0=ps,
                        scalar=ISW, in1=bvV_sb[:, nh * 512:(nh + 1) * 512],
                        op0=AluOp.mult, op1=AluOp.add)
        fr_x1T()

        # prefetch proj + GRU1 r-gate weights (SP queue, before attention)
        wprp = root.enter_context(tc.tile_pool(name="wprp", bufs=1, side="left"))
        wproj = wprp.tile([P, DC, D], FP8)
        nc.sync.dma_start(out=wproj, in_=wproj_d[:].rearrange("(kc p) n -> p kc n", p=P))
        wr1 = wprp.tile([P, DC, D], FP8)
        nc.sync.dma_start(out=wr1, in_=gw_d[(1, "Wr")][:].rearrange("(kc p) n -> p kc n", p=P))
        ur1 = wprp.tile([P, DC, D], FP8)
        nc.sync.dma_start(out=ur1, in_=gw_d[(1, "Ur")][:].rearrange("(kc p) n -> p kc n", p=P))

        # reserve GRU output tiles below the inp tiles (LIFO frees)
        o1T_f, fr_o1f = mk("o1T_f", (P, DC, CUR), F32, "left")
        o1_8, fr_o18 = mk("o1_8", (P, DC, CUR), FP8, "left")

        # load GRU1 inputs early (SP queue; needed in phase 4)
        inpT_f, fr_inpf = mk("inpT_f", (P, DC, CUR), F32, "left")
        inp_8, fr_inp8 = mk("inp_8", (P, DC, CUR), FP8, "left")
        nc.sync.dma_start(out=inpT_f, in_=inpT_d[:].rearrange("(kc p) t -> p kc t", p=P))
        for n in range(DC):
            VE(n).tensor_copy(inp_8[:, n, :], inpT_f[:, n, :])

        # ================= Phase 3: attention =================
        avT, fr_avT = mk("avT", (P, DC, CUR), FP8, "left")
        with ExitStack() as ph:
            scp = ph.enter_context(tc.tile_pool(name="scp", bufs=3, space="PSUM"))
            ptp = ph.enter_context(tc.tile_pool(name="ptp", bufs=1, space="PSUM"))
            avp = ph.enter_context(tc.tile_pool(name="avp", bufs=1, space="PSUM"))
            pbw = ph.enter_context(tc.tile_pool(name="pbw", bufs=2, side="left"))
            shw = ph.enter_context(tc.tile_pool(name="shw", bufs=9, side="left"))
            smw = ph.enter_context(tc.tile_pool(name="smw", bufs=3, side="left"))
            enw = ph.enter_context(tc.tile_pool(name="enw", bufs=3, side="left"))
            atw = ph.enter_context(tc.tile_pool(name="atw", bufs=2, side="left"))
            dnw = ph.enter_context(tc.tile_pool(name="dnw", bufs=3, side="left"))

            def hsl(t, h):
                ch, rb = h // 2, (h % 2) * HEAD_DIM
                return t[rb:rb + HEAD_DIM, ch, :]

            def head_pos(h):
                """Pos scores + shift round trip for head h."""
                qvh = hsl(qvT, h)
                rh = hsl(rT, h)
                s_t = scr[h % n_scr]
                pb = pbw.tile([P, TCC, 1024], FP8, name="pb")
                for ic in range(TCC - 1):
                    nc.gpsimd.memset(pb[:, ic, 0:384 - 128 * ic], 0.0)
                for ic in range(TCC):
                    c0 = 384 - 128 * ic          # first rel col needed
                    pp = scp.tile([P, 1024], F32, name="pp", tag="sc")
                    nc.tensor.matmul(pp[:, c0:512], lhsT=qvh[:, ic * P:(ic + 1) * P],
                                     rhs=rh[:, c0:512], start=True, stop=True)
                    nc.tensor.matmul(pp[:, 512:1024], lhsT=qvh[:, ic * P:(ic + 1) * P],
                                     rhs=rh[:, 512:1024], start=True, stop=True)
                    if ic % 2 == 0:
                        nc.scalar.copy(pb[:, ic, c0:1024], pp[:, c0:1024])
                    else:
                        nc.vector.tensor_copy(pb[:, ic, c0:1024], pp[:, c0:1024])
                # one combined scratch write for all 4 query chunks
                nc.sync.dma_start(out=s_t[:, :, 0:1024], in_=pb)
                shps = []
                for ic in range(TCC):
                    wr = (ic + 5) * 128          # shifted-read width
                    shp = shw.tile([P, FULL], FP8, name="shp")
                    shift_ap = bass.AP(tensor=s_t.tensor,
                                       offset=s_t.offset + 1536 * ic + 511 - 128 * ic,
                                       ap=[[TCC * 1536 - 1, P], [1, wr]])
                    nc.sync.dma_start(out=shp[:, 0:wr], in_=shift_ap)
                    shps.append(shp)
                return shps

            def head_content(h, shps):
                """Content scores + softmax numerators for head h."""
                quh = hsl(quT, h)
                kh = hsl(kT, h)
                esn = enw.tile([P, TCC, FULL], BF16, name="esn")
                den = dnw.tile([P, TCC], F32, name="den")
                rec = dnw.tile([P, TCC], F32, name="rec")
                for ic in range(TCC):
                    wr = (ic + 5) * 128
                    shp = shps[ic]
                    cp = scp.tile([P, 1024], F32, name="cp", tag="sc")
                    nc.tensor.matmul(cp[:, 0:512], lhsT=quh[:, ic * P:(ic + 1) * P],
                                     rhs=kh[:, 0:512], start=True, stop=False)
                    nc.tensor.matmul(cp[:, 512:wr], lhsT=quh[:, ic * P:(ic + 1) * P],
                                     rhs=kh[:, 512:wr], start=True, stop=False)
                    # add shifted pos scores in PSUM: cp += I^T @ shp
                    nc.tensor.matmul(cp[:, 0:512], lhsT=ident_8,
                                     rhs=shp[:, 0:512], start=False, stop=True)
                    nc.tensor.matmul(cp[:, 512:wr], lhsT=ident_8,
                                     rhs=shp[:, 512:wr], start=False, stop=True)
                    nc.scalar.activation(out=esn[:, ic, 0:wr], in_=cp[:, 0:wr],
                                         func=Act.Exp, scale=SCALE,
                                         accum_out=den[:, ic:ic + 1])
                    nc.vector.reciprocal(out=rec[:, ic:ic + 1],
                                         in_=den[:, ic:ic + 1])
                    nc.gpsimd.tensor_scalar(out=esn[:, ic, 0:wr],
                                            in0=esn[:, ic, 0:wr],
                                            scalar1=rec[:, ic:ic + 1],
                                            scalar2=ES_S, op0=AluOp.mult,
                                            op1=AluOp.mult)
                return esn, rec

            def head_tail(h, esn, rec):
                """Transpose + AV + normalize for head h (two heads behind)."""
                ch, rb = h // 2, (h % 2) * HEAD_DIM
                attnT = atw.tile([P, TCF, 512], FP8, name="attnT")
                nc.gpsimd.memset(attnT[:, 5, 0:128], 0.0)
                nc.gpsimd.memset(attnT[:, 7, 256:384], 0.0)
                for jc in range(TCF):
                    ic0 = max(0, jc - 4)
                    pt = ptp.tile([P, 512], BF16, name="pt", tag="pt")
                    for ic in range(ic0, TCC):
                        nc.tensor.transpose(pt[:, ic * P:(ic + 1) * P],
                                            esn[:, ic, jc * P:(jc + 1) * P],
                                            ident_b)
                    nc.vector.tensor_copy(attnT[:, jc, ic0 * P:512],
                                          pt[:, ic0 * P:512])
                av = avp.tile([P, 512], F32, name="av", tag="av")
                for qh in range(2):
                    pairs = [0, 1, 2] if qh == 0 else [0, 1, 2, 3]
                    for i, pr in enumerate(pairs):
                        nc.tensor.matmul(
                            av[0:HEAD_DIM, qh * 256:qh * 256 + 256],
                            lhsT=v_nat[:, 2 * pr:2 * pr + 2,
                                       h * HEAD_DIM:(h + 1) * HEAD_DIM],
                            rhs=attnT[:, 2 * pr:2 * pr + 2,
                                      qh * 256:qh * 256 + 256],
                            perf_mode=DR,
                            start=(i == 0), stop=(i == len(pairs) - 1))
                nc.vector.tensor_scalar_mul(avT[rb:rb + HEAD_DIM, ch, :],
                                            av[0:HEAD_DIM, :], IES)

            # depth-2 software pipeline: pos(h) | content(h-1) | tail(h-2)
            sh_st = {}
            cr_st = {}
            for it in range(HEAD_NUM + 2):
                if it < HEAD_NUM:
                    sh_st[it] = head_pos(it)
                if 1 <= it <= HEAD_NUM:
                    cr_st[it - 1] = head_content(it - 1, sh_st.pop(it - 1))
                if it >= 2:
                    esn, rec = cr_st.pop(it - 2)
                    head_tail(it - 2, esn, rec)
        fr_qvT(); fr_quT(); fr_rT(); fr_v(); fr_kT()

        # ================= Phase 4: proj + GRU1 =================
        psum_box["p"] = root.enter_context(
            tc.tile_pool(name="psum_d", bufs=4, space="PSUM"))
        psum_box["s"] = root.enter_context(
            tc.tile_pool(name="psum_sd", bufs=2, space="PSUM"))
        a1T, fr_a1T = mk("a1T", (P, DC, CUR), FP8, "right")
        for n in range(DC):
            ps = PS()
            _dr_quads(nc, ps, [(wproj, avT, n * P, 0, KP)])
            nc.scalar.activation(out=a1T[:, n, :], in_=ps, func=Act.Relu,
                                 scale=ISW, bias=bproj_sb[:, n:n + 1])
        fr_avT()

        with ExitStack() as ph:
            _gru(nc, tc, ph, PS, gw_d, 1, a1T, inp_8, inpT_f, nbg1_sb,
                 o1T_f, o1_8, VE, pre=(wr1, ur1))
        fr_inp8(); fr_inpf(); fr_a1T()

        # ================= Phase 5: LN2 =================
        x2T, fr_x2T = mk("x2T", (P, DC, CUR), FP8, "right")
        with ExitStack() as ph:
            lw = ph.enter_context(tc.tile_pool(name="lw", bufs=2, side="left"))
            sqp = ph.enter_context(tc.tile_pool(name="sqp", bufs=1, side="left"))
            sq = sqp.tile([P, DC, 512], FP8, name="sq")
            for n in range(DC):
                VE(n).tensor_mul(sq[:, n, :], o1_8[:, n, :], o1_8[:, n, :])
            s1 = SM()
            for n in range(DC):
                nc.tensor.matmul(s1, lhsT=ones_red8, rhs=o1_8[:, n, :],
                                 start=(n == 0), stop=(n == DC - 1))
            mean = lw.tile([1, 512], F32, name="mean")
            nc.vector.tensor_scalar_mul(mean, s1, 1.0 / D)
            s2 = SM()
            for n in range(DC):
                nc.tensor.matmul(s2, lhsT=ones_red8, rhs=sq[:, n, :],
                                 start=(n == 0), stop=(n == DC - 1))
            m2m = lw.tile([1, 512], F32, name="m2m")
            nc.vector.tensor_scalar_mul(m2m, s2, 1.0 / D)
            var = lw.tile([1, 512], F32, name="var")
            nc.vector.scalar_tensor_tensor(out=var, in0=mean, scalar=1.0,
                                           in1=mean, op0=AluOp.mult,
                                           op1=AluOp.mult)
            nc.vector.tensor_sub(var, m2m, var)
            sd = lw.tile([1, 512], F32, name="sd2")
            nc.scalar.activation(out=sd, in_=var, func=Act.Sqrt,
                                 bias=eps_t[0:1, :])
            rstd = lw.tile([1, 512], F32, name="rstd2")
            nc.vector.reciprocal(out=rstd, in_=sd)
            meanB = lw.tile([P, 512], F32, name="meanB")
            nc.gpsimd.partition_broadcast(meanB, mean)
            rstdB = lw.tile([P, 512], F32, name="rstdB")
            nc.gpsimd.partition_broadcast(rstdB, rstd)
            for n in range(DC):
                t1 = lw.tile([P, 512], F32, name="t1")
                VE(n).tensor_sub(t1, o1T_f[:, n, :], meanB)
                VE(n).tensor_mul(t1, t1, rstdB)
                VE(n + 1).tensor_scalar(out=x2T[:, n, :], in0=t1,
                                        scalar1=ln2g_sb[:, n:n + 1],
                                        scalar2=ln2b_sb[:, n:n + 1],
                                        op0=AluOp.mult, op1=AluOp.add)

        # ================= Phase 6: MLP =================
        with ExitStack() as ph6:
            m1w = ph6.enter_context(tc.tile_pool(name="m1w", bufs=1, side="right"))
            m1T = m1w.tile([P, HC, 512], FP8)
            w12p = ph6.enter_context(tc.tile_pool(name="w12p", bufs=1, side="right"))
            w1t = w12p.tile([P, DC, HID], FP8, name="w1t")
            nc.sync.dma_start(out=w1t, in_=w1_d[:].rearrange("(kc p) n -> p kc n", p=P))
            w2t = w12p.tile([P, HC, D], FP8, name="w2t")
            nc.sync.dma_start(out=w2t, in_=w2_d[:].rearrange("(kc p) n -> p kc n", p=P))
            for n in range(HC):
                ps = PS()
                _dr_quads(nc, ps, [(w1t, x2T, n * P, 0, KP)])
                nc.scalar.activation(out=m1T[:, n, :], in_=ps, func=Act.Relu,
                                     scale=ISW, bias=b1_sb[:, n:n + 1])
            m2T, fr_m2T = mk("m2T", (P, DC, CUR), FP8, "left")
            KPH = list(range(HC // 2))
            for n in range(DC):
                ps = PS()
                _dr_quads(nc, ps, [(w2t, m1T, n * P, 0, KPH)])
                nc.scalar.activation(out=m2T[:, n, :], in_=ps, func=Act.Relu,
                                     scale=ISW, bias=b2_sb[:, n:n + 1])
        fr_x2T()

        # ================= Phase 7: GRU2 + output =================
        o2T_f, fr_o2 = mk("o2T_f", (P, DC, CUR), F32, "right")
        o2r = out_d[:].rearrange("(kc p) t -> p kc t", p=P)

        def out_chunk(n):
            nc.sync.dma_start(out=o2r[:, n, :], in_=o2T_f[:, n, :])

        with ExitStack() as ph:
            _gru(nc, tc, ph, PS, gw_d, 2, m2T, o1_8, o1T_f, nbg2_sb,
                 o2T_f, None, VE, on_chunk=out_chunk)
        fr_m2T(); fr_o18(); fr_o1f()
        fr_o2()


def _gru(nc, tc, ph, PS, gw_d, g, yT, x8, xf, nbg_sb, oT_f, o_8, VE,
         pre=None, on_chunk=None):
    gwp = ph.enter_context(tc.tile_pool(name=f"gw{g}", bufs=4, side="left"))
    gtmp = ph.enter_context(tc.tile_pool(name=f"gt{g}", bufs=3, side="left"))
    gper = ph.enter_context(tc.tile_pool(name=f"gp{g}", bufs=1, side="left"))
    KP = [0, 1, 2, 3]

    def loadw(m):
        w = gwp.tile([P, DC, D], FP8, name=f"gwt_{m}", tag="gwt")
        nc.sync.dma_start(out=w, in_=gw_d[(g, m)][:].rearrange("(kc p) n -> p kc n", p=P))
        return w

    wr, ur = pre if pre is not None else (loadw("Wr"), loadw("Ur"))
    wz, uz = loadw("Wz"), loadw("Uz")
    rx = gper.tile([P, DC, 512], FP8, name="rx")
    for n in range(DC):
        ps = PS()
        _dr_quads(nc, ps, [(wr, yT, n * P, 0, KP), (ur, x8, n * P, 0, KP)])
        rr = gtmp.tile([P, 512], F32, name="rr")
        nc.scalar.activation(out=rr, in_=ps, func=Act.Sigmoid, scale=ISW)
        VE(n).tensor_mul(rx[:, n, :], rr, xf[:, n, :])
    wg, ug = loadw("Wg"), loadw("Ug")
    zt = gper.tile([P, DC, 512], BF16, name="zt")
    for n in range(DC):
        ps = PS()
        _dr_quads(nc, ps, [(wz, yT, n * P, 0, KP), (uz, x8, n * P, 0, KP)])
        nc.scalar.activation(out=zt[:, n, :], in_=ps, func=Act.Sigmoid,
                             scale=ISW, bias=nbg_sb[:, n:n + 1])
    for n in range(DC):
        ps = PS()
        _dr_quads(nc, ps, [(wg, yT, n * P, 0, KP), (ug, rx, n * P, 0, KP)])
        ht = gtmp.tile([P, 512], F32, name="ht")
        nc.scalar.activation(out=ht, in_=ps, func=Act.Tanh, scale=ISW)
        VE(n).tensor_sub(ht, ht, xf[:, n, :])
        VE(n + 1).tensor_mul(ht, ht, zt[:, n, :])
        VE(n).tensor_add(oT_f[:, n, :], ht, xf[:, n, :])
        if o_8 is not None:
            VE(n + 1).tensor_copy(o_8[:, n, :], oT_f[:, n, :])
        if on_chunk is not None:
            on_chunk(n)


_NC_CACHE = {}


def _get_nc():
    if "nc" not in _NC_CACHE:
        _NC_CACHE["nc"] = _build()
    return _NC_CACHE["nc"]


def _chunk_t(vec):
    n = vec.shape[0] // P
    return np.ascontiguousarray(vec.reshape(n, P).T.astype(np.float32))


def _fp8w(w):
    f8 = ml_dtypes.float8_e4m3
    return np.clip(np.asarray(w, np.float32) * WS, -240.0, 240.0).astype(f8)


def _prep(inputs):
    f32 = np.float32
    f8 = ml_dtypes.float8_e4m3
    inp = np.asarray(inputs["inputs"], f32)
    mem = np.asarray(inputs["memory"], f32)
    pos = np.asarray(inputs["pos_embedding"], f32)[:, 0, :]
    wkv = np.asarray(inputs["Wkv"], f32)

    shared = {
        "posT8": np.clip(np.ascontiguousarray(pos.T), -240, 240).astype(f8),
        "u_t": _chunk_t(np.asarray(inputs["u"], f32).reshape(-1)),
        "v_t": _chunk_t(np.asarray(inputs["v"], f32).reshape(-1)),
        "ln1_g_t": _chunk_t(np.asarray(inputs["ln1_g"], f32)),
        "ln1_b_t": _chunk_t(np.asarray(inputs["ln1_b"], f32)),
        "ln2_g_t": _chunk_t(np.asarray(inputs["ln2_g"], f32)),
        "ln2_b_t": _chunk_t(np.asarray(inputs["ln2_b"], f32)),
        "bkvK_t": _chunk_t(np.asarray(inputs["bkv"], f32)[0:D]),
        "bkvV_row": np.asarray(inputs["bkv"], f32)[D:2 * D].reshape(1, D),
        "bq_t": _chunk_t(np.asarray(inputs["bq"], f32)),
        "bpos_t": _chunk_t(np.asarray(inputs["bpos"], f32)),
        "bproj_t": _chunk_t(np.asarray(inputs["bproj"], f32)),
        "b1_t": _chunk_t(np.asarray(inputs["mlp_b1"], f32)),
        "b2_t": _chunk_t(np.asarray(inputs["mlp_b2"], f32)),
        "nbg1_t": _chunk_t(-np.asarray(inputs["g1_bg"], f32)),
        "nbg2_t": _chunk_t(-np.asarray(inputs["g2_bg"], f32)),
        "WkvK8": _fp8w(wkv[:, 0:D]),
        "WkvV8": _fp8w(wkv[:, D:2 * D]),
        "Wq8": _fp8w(inputs["Wq"]),
        "Wpos8": _fp8w(inputs["Wpos"]),
        "Wproj8": _fp8w(inputs["Wproj"]),
        "mlp_W18": _fp8w(inputs["mlp_W1"]),
        "mlp_W28": _fp8w(inputs["mlp_W2"]),
    }
    for g in (1, 2):
        for m in ("Wr", "Ur", "Wz", "Uz", "Wg", "Ug"):
            shared[f"g{g}_{m}8"] = _fp8w(inputs[f"g{g}_{m}"])

    in_maps = []
    for b in range(BS):
        im = dict(shared)
        im["x_full"] = np.ascontiguousarray(
            np.concatenate([mem[:, b, :], inp[:, b, :]], axis=0).T)
        im["inpT"] = np.ascontiguousarray(inp[:, b, :].T)
        in_maps.append(im)
    return in_maps


def _post(out_t):
    """Device output is [D, CUR]; transpose to [CUR, D]."""
    return np.ascontiguousarray(np.asarray(out_t).T.astype(np.float32))


def kernel(**inputs):
    nc = _get_nc()
    in_maps = _prep(inputs)
    res = run_bass_kernel_spmd(nc, in_maps, core_ids=list(range(BS)))
    out = np.stack([_post(res.results[b]["out"]) for b in range(BS)], axis=1)
    return np.ascontiguousarray(out.astype(np.float32))


if __name__ == "__main__":
    _get_nc()
    print("build+compile OK")


# revision 61
# speedup vs baseline: 1.0030x; 1.0030x over previous
"""GTrXL layer (TransformerXL attention + GRU gating) on 8 TRN2 NeuronCores.

Sharding: pure data-parallel over batch (BS=8 -> 1 batch element per core).
No collectives. Per-core Bass/Tile kernel computes the full layer for its
batch element.

Layout convention on-chip: activations are kept TRANSPOSED [feature, token]
(feature on partitions, 128-chunks).

Precision strategy: all dense matmuls run in fp8-e4m3 with DoubleRow perf
mode (2 contraction k-tiles per pass, full 128-wide stationary -> psum
[128, 256] per 128-cycle pass). Weights are scaled by 256 on the host before
fp8 quantization (keeps values out of the subnormal range); every PSUM
consumer applies a 2^-8 scale (alternating DVE tensor_scalar / ACT
activation(Identity) — only those engines may touch PSUM; GPSIMD cannot).
Activations quantize to fp8 at natural scale. Elementwise math (LN, GRU
combine) stays f32; logits/es stay bf16.

LN1 runs feature-major on a host-transposed input: mean/E[x^2] via
ones-matmul partition reductions (bf16), then normalize + fp8 quantize.

Relative-shift: per head, pos scores P[i, rel] for the needed rel range
[384-128*ic, 1024) of all 4 query chunks are written in one DMA to a 4-row
DRAM scratch (row stride 1536, fp8, tail 512 cols pre-filled with -240).
The shifted read  shifted[i, j] = P[i, 511 - 128*ic + j - i]  is a strided
DMA (row step 4*1536-1, sub-row offset 511-128*ic — the correct global TrXL
shift; the staged baseline used the local chunk index here, which is subtly
wrong but heavily damped by the GRU gates). The -240 pad lands exactly on
the masked region and exp()s to ~0.

Attention is a depth-2 software pipeline over heads: pos+shift(h) |
content+exp(h-1) | transpose+AV(h-2). The shifted pos scores are added to
the content logits IN PSUM by an identity-matmul accumulate (PE), exp reads
PSUM directly and emits softmax denominators via accum_out; normalization is
folded into the es tile on GPSIMD (SBUF-only) with a 128x fp8 scale, and the
AV output is rescaled by 2^-7.
"""

import sys

if '/opt/trn_rl_repo' not in sys.path:
    sys.path.insert(0, '/opt/trn_rl_repo')

import numpy as np
import ml_dtypes

import concourse.bass as bass
import concourse.tile as tile
from concourse import bacc, mybir
from concourse.bass_utils import run_bass_kernel_spmd
from concourse.masks import make_identity

BF16 = mybir.dt.bfloat16
F32 = mybir.dt.float32
FP8 = mybir.dt.float8e4

HEAD_NUM, HEAD_DIM = 16, 64
D, HID = 1024, 4096
CUR, PREV, BS = 512, 512, 8
FULL = CUR + PREV
EPS = 1e-5
SCALE = 1.0 / (HEAD_DIM ** 0.5)
P = 128
DC = D // P          # 8 feature chunks
HC = HID // P        # 32 hidden chunks
TCF = FULL // P      # 8 full-token chunks
TCC = CUR // P       # 4 query-token chunks
NEG = -1.0e30
WS = 256.0           # host-side weight scale before fp8 quantization
ISW = 1.0 / WS       # psum de-scale
ES_S = 128.0         # softmax-weight fp8 scale
IES = 1.0 / ES_S

AluOp = mybir.AluOpType
Act = mybir.ActivationFunctionType
DR = mybir.MatmulPerfMode.DoubleRow


def _dram_in(dram, name, shape, dtype):
    return dram.tile(list(shape), dtype, kind="ExternalInput", name=name,
                     uniquify=False)


def _dr_quads(nc, ps, segs):
    """Fill psum [128, 512] via DoubleRow chains (full 128-wide stationary).

    segs: list of (w, x, c0, t0, kpairs) — accumulate over all segs:
      ps[n, t] += sum_k w[k, c0+n] * x[k, t0+t]   (k over kpairs*256 lanes)
    w, x are [P, 2*kpairs.., *] fp8 tiles (chunk dim second).
    """
    for qh in range(2):
        out = ps[:, qh * 256:qh * 256 + 256]
        total = sum(len(s[4]) for s in segs)
        i = 0
        for (w, x, c0, t0, kpairs) in segs:
            for m in kpairs:
                nc.tensor.matmul(
                    out,
                    lhsT=w[:, 2 * m:2 * m + 2, c0:c0 + P],
                    rhs=x[:, 2 * m:2 * m + 2,
                          t0 + qh * 256:t0 + qh * 256 + 256],
                    perf_mode=DR,
                    start=(i == 0), stop=(i == total - 1))
                i += 1


def _build():
    nc = bacc.Bacc("TRN2", target_bir_lowering=False)
    with tile.TileContext(nc) as tc:
        _emit(nc, tc)
    nc.compile()
    return nc


def _emit(nc, tc):
    from contextlib import ExitStack

    with ExitStack() as root:
        dram = root.enter_context(tc.tile_pool(name="io", bufs=1, space="DRAM"))

        # ---------------- DRAM I/O ----------------
        x_full = _dram_in(dram, "x_full", (D, FULL), F32)
        inpT_d = _dram_in(dram, "inpT", (D, CUR), F32)
        posT_d = _dram_in(dram, "posT8", (D, FULL), FP8)
        u_d = _dram_in(dram, "u_t", (P, DC), F32)
        v_d = _dram_in(dram, "v_t", (P, DC), F32)
        ln1g_d = _dram_in(dram, "ln1_g_t", (P, DC), F32)
        ln1b_d = _dram_in(dram, "ln1_b_t", (P, DC), F32)
        ln2g_d = _dram_in(dram, "ln2_g_t", (P, DC), F32)
        ln2b_d = _dram_in(dram, "ln2_b_t", (P, DC), F32)
        bkvK_d = _dram_in(dram, "bkvK_t", (P, DC), F32)
        bkvV_d = _dram_in(dram, "bkvV_row", (1, D), F32)
        bq_d = _dram_in(dram, "bq_t", (P, DC), F32)
        bpos_d = _dram_in(dram, "bpos_t", (P, DC), F32)
        bproj_d = _dram_in(dram, "bproj_t", (P, DC), F32)
        b1_d = _dram_in(dram, "b1_t", (P, HC), F32)
        b2_d = _dram_in(dram, "b2_t", (P, DC), F32)
        nbg1_d = _dram_in(dram, "nbg1_t", (P, DC), F32)
        nbg2_d = _dram_in(dram, "nbg2_t", (P, DC), F32)

        wkvK_d = _dram_in(dram, "WkvK8", (D, D), FP8)
        wkvV_d = _dram_in(dram, "WkvV8", (D, D), FP8)
        wq_d = _dram_in(dram, "Wq8", (D, D), FP8)
        wpos_d = _dram_in(dram, "Wpos8", (D, D), FP8)
        wproj_d = _dram_in(dram, "Wproj8", (D, D), FP8)
        gw_d = {}
        for g in (1, 2):
            for m in ("Wr", "Ur", "Wz", "Uz", "Wg", "Ug"):
                gw_d[(g, m)] = _dram_in(dram, f"g{g}_{m}8", (D, D), FP8)
        w1_d = _dram_in(dram, "mlp_W18", (D, HID), FP8)
        w2_d = _dram_in(dram, "mlp_W28", (HID, D), FP8)

        # transposed output [D, CUR]; host transposes back
        out_d = dram.tile([D, CUR], F32, kind="ExternalOutput", name="out",
                          uniquify=False)

        # per-head scratch: 4 sub-rows (one per query chunk) of width 1536,
        # fp8, tail 512 cols pre-filled with -240 (acts as -inf after exp)
        n_scr = 4
        scr = [dram.tile([P, TCC, 1536], FP8, name=f"scr{s}")
               for s in range(n_scr)]

        # ---------------- constants ----------------
        const = root.enter_context(tc.tile_pool(name="const", bufs=1))
        ident_f = const.tile([P, P], F32)
        make_identity(nc, ident_f)
        ident_b = const.tile([P, P], BF16)
        make_identity(nc, ident_b)
        ident_8 = const.tile([P, P], FP8)
        make_identity(nc, ident_8)
        ones_red8 = const.tile([P, 1], FP8)
        nc.vector.memset(ones_red8, 1.0)
        ones_b = const.tile([P, 1], BF16)
        nc.vector.memset(ones_b, 1.0)
        eps_t = const.tile([P, 1], F32)
        nc.vector.memset(eps_t, EPS)

        def cload(name, dref, shape, dtype=F32):
            t = const.tile(list(shape), dtype, name=name)
            nc.sync.dma_start(out=t, in_=dref[:])
            return t

        u_sb = cload("u_sb", u_d, (P, DC))
        v_sb = cload("v_sb", v_d, (P, DC))
        ln2g_sb = cload("ln2g_sb", ln2g_d, (P, DC))
        ln2b_sb = cload("ln2b_sb", ln2b_d, (P, DC))
        bkvK_sb = cload("bkvK_sb", bkvK_d, (P, DC))
        bq_sb = cload("bq_sb", bq_d, (P, DC))
        bpos_sb = cload("bpos_sb", bpos_d, (P, DC))
        bproj_sb = cload("bproj_sb", bproj_d, (P, DC))
        b1_sb = cload("b1_sb", b1_d, (P, HC))
        b2_sb = cload("b2_sb", b2_d, (P, DC))
        nbg1_sb = cload("nbg1_sb", nbg1_d, (P, DC))
        nbg2_sb = cload("nbg2_sb", nbg2_d, (P, DC))
        ln1g_sb = cload("ln1g_sb", ln1g_d, (P, DC))
        ln1b_sb = cload("ln1b_sb", ln1b_d, (P, DC))
        # V bias broadcast to all partitions (free-dim varying)
        bvV_sb = const.tile([P, D], F32, name="bvV_sb")
        nc.sync.dma_start(out=bvV_sb, in_=bass.AP(
            tensor=bkvV_d.tensor, offset=bkvV_d.offset, ap=[[0, P], [1, D]]))

        padw = const.tile([P, TCC, 512], FP8)
        nc.vector.memset(padw, -240.0)
        for s in range(n_scr):
            nc.sync.dma_start(out=scr[s][:, :, 1024:1536], in_=padw)

        # engine rotation for copies / elementwise
        vecs = [nc.vector, nc.gpsimd]

        def VE(i):
            return vecs[i % 2]

        def psum_sb(i, out, ps, bias_ap):
            """out = ps * ISW + bias  (PSUM-legal engines only)."""
            if i % 2 == 0:
                nc.vector.tensor_scalar(out=out, in0=ps, scalar1=ISW,
                                        scalar2=bias_ap, op0=AluOp.mult,
                                        op1=AluOp.add)
            else:
                nc.scalar.activation(out=out, in_=ps, func=Act.Identity,
                                     scale=ISW, bias=bias_ap)

        # phase-scoped psum pools (PSUM is only 8 banks; attention needs them)
        psum_box = {}

        def PS():
            return psum_box["p"].tile([P, 512], F32, name="ps", tag="ps")

        def SM():
            return psum_box["s"].tile([1, 512], F32, name="sm", tag="sm")

        def mk(name, shape, dtype, side):
            t, fr = tc.tile(list(shape), dtype, name=name, side=side)
            return t, fr

        x1T8, fr_x1T = mk("x1T8", (P, DC, FULL), FP8, "left")

        # ============ Phase 1+2 fused: rT first (warms PE), then LN1 with
        # ============ V/kT/q matmuls interleaved as token chunks complete
        kT, fr_kT = mk("kT", (P, DC, FULL), FP8, "right")
        v_nat, fr_v = mk("v_nat", (P, TCF, D), FP8, "right")
        rT, fr_rT = mk("rT", (P, DC, FULL), FP8, "right")
        quT, fr_quT = mk("quT", (P, DC, CUR), FP8, "right")
        qvT, fr_qvT = mk("qvT", (P, DC, CUR), FP8, "right")

        KP = [0, 1, 2, 3]  # the 4 k-chunk pairs covering D=1024

        with ExitStack() as ph:
            psum_box["p"] = ph.enter_context(
                tc.tile_pool(name="psum12", bufs=4, space="PSUM"))
            wkvp = ph.enter_context(tc.tile_pool(name="wkvp", bufs=1, side="right"))
            wpos = wkvp.tile([P, DC, D], FP8)
            nc.scalar.dma_start(out=wpos, in_=wpos_d[:].rearrange("(kc p) n -> p kc n", p=P))
            posT_sb = wkvp.tile([P, DC, FULL], FP8)
            nc.scalar.dma_start(out=posT_sb, in_=posT_d[:].rearrange("(kc p) f -> p kc f", p=P))
            wkvK = wkvp.tile([P, DC, D], FP8)
            nc.scalar.dma_start(out=wkvK, in_=wkvK_d[:].rearrange("(kc p) n -> p kc n", p=P))
            wkvV = wkvp.tile([P, DC, D], FP8)
            nc.scalar.dma_start(out=wkvV, in_=wkvV_d[:].rearrange("(kc p) n -> p kc n", p=P))
            wq = wkvp.tile([P, DC, D], FP8)
            nc.scalar.dma_start(out=wq, in_=wq_d[:].rearrange("(kc p) n -> p kc n", p=P))

            # rT only needs pos inputs — keeps the PE busy during LN1
            for n in range(DC):
                for fh in range(2):
                    ps = PS()
                    _dr_quads(nc, ps, [(wpos, posT_sb, n * P, fh * 512, KP)])
                    psum_sb(n + fh, rT[:, n, fh * 512:(fh + 1) * 512],
                            ps, bpos_sb[:, n:n + 1])

            # ---- feature-major LN1: x arrives pre-transposed [D, FULL] ----
            lnw = ph.enter_context(tc.tile_pool(name="lnw", bufs=1, side="left"))
            xT_f = lnw.tile([P, DC, FULL], F32, name="xT_f")
            xr = x_full[:].rearrange("(kc p) t -> p kc t", p=P)
            for k in range(DC):
                nc.sync.dma_start(out=xT_f[:, k, :], in_=xr[:, k, :])
            xb = lnw.tile([P, DC, FULL], BF16, name="xb")
            sq_b = lnw.tile([P, DC, FULL], BF16, name="sq_b")
            rowp = ph.enter_context(tc.tile_pool(name="rowp", bufs=4, space="PSUM"))
            s_mean = [rowp.tile([1, 512], F32, name="s_mean", tag="row")
                      for _ in range(2)]
            s_sq = [rowp.tile([1, 512], F32, name="s_sq", tag="row")
                    for _ in range(2)]
            for k in range(DC):
                nc.gpsimd.tensor_copy(xb[:, k, :], xT_f[:, k, :])
                VE(k).tensor_mul(sq_b[:, k, :], xb[:, k, :], xb[:, k, :])
            for fh in range(2):
                for k in range(DC):
                    nc.tensor.matmul(
                        s_mean[fh], lhsT=ones_b,
                        rhs=xb[:, k, fh * 512:(fh + 1) * 512],
                        start=(k == 0), stop=(k == DC - 1))
                for k in range(DC):
                    nc.tensor.matmul(
                        s_sq[fh], lhsT=ones_b,
                        rhs=sq_b[:, k, fh * 512:(fh + 1) * 512],
                        start=(k == 0), stop=(k == DC - 1))
            mrow = lnw.tile([1, FULL], F32, name="mrow")
            vrow = lnw.tile([1, FULL], F32, name="vrow")
            for fh in range(2):
                sl = slice(fh * 512, (fh + 1) * 512)
                nc.vector.tensor_scalar_mul(mrow[:, sl], s_mean[fh], 1.0 / D)
                nc.vector.tensor_scalar_mul(vrow[:, sl], s_sq[fh], 1.0 / D)
            # var = E[x^2] - mean^2 ; rstd = 1/sqrt(var+eps)
            m2row = lnw.tile([1, FULL], F32, name="m2row")
            nc.vector.tensor_mul(m2row, mrow, mrow)
            nc.vector.tensor_sub(vrow, vrow, m2row)
            srow = lnw.tile([1, FULL], F32, name="srow")
            nc.scalar.activation(out=srow, in_=vrow, func=Act.Sqrt,
                                 bias=eps_t[0:1, :])
            rrow = lnw.tile([1, FULL], F32, name="rrow")
            nc.vector.reciprocal(out=rrow, in_=srow)
            meanB = lnw.tile([P, FULL], F32, name="meanB")
            nc.gpsimd.partition_broadcast(meanB, mrow)
            rstdB = lnw.tile([P, FULL], F32, name="rstdB")
            nc.gpsimd.partition_broadcast(rstdB, rrow)
            tw = ph.enter_context(tc.tile_pool(name="tw", bufs=3, side="left"))
            for k in range(DC):
                t1 = tw.tile([P, FULL], F32, name="t1")
                VE(k).tensor_sub(t1, xT_f[:, k, :], meanB)
                VE(k).tensor_mul(t1, t1, rstdB)
                VE(k + 1).tensor_scalar(out=x1T8[:, k, :], in0=t1,
                                        scalar1=ln1g_sb[:, k:k + 1],
                                        scalar2=ln1b_sb[:, k:k + 1],
                                        op0=AluOp.mult, op1=AluOp.add)

            # ---- kv / q projections ----
            qw = ph.enter_context(tc.tile_pool(name="qw", bufs=3, side="left"))
            for n in range(DC):
                ps = PS()
                _dr_quads(nc, ps, [(wq, x1T8, n * P, CUR, KP)])
                qn = qw.tile([P, 512], F32, name="qn")
                nc.vector.tensor_scalar(out=qn, in0=ps, scalar1=ISW,
                                        scalar2=bq_sb[:, n:n + 1],
                                        op0=AluOp.mult, op1=AluOp.add)
                nc.vector.tensor_scalar_add(quT[:, n, :], qn,
                                            u_sb[:, n:n + 1])
                nc.gpsimd.tensor_scalar_add(qvT[:, n, :], qn,
                                            v_sb[:, n:n + 1])
            for th in range(2):
                for n in range(DC):
                    ps = PS()
                    _dr_quads(nc, ps, [(wkvK, x1T8, n * P, th * 512, KP)])
                    psum_sb(n + th, kT[:, n, th * 512:(th + 1) * 512],
                            ps, bkvK_sb[:, n:n + 1])
            for tcx in range(TCF):
                for nh in range(2):
                    ps = PS()
                    _dr_quads(nc, ps, [(x1T8, wkvV, tcx * P, nh * 512, KP)])
                    nc.vector.scalar_tensor_tensor(
                        out=v_nat[:, tcx, nh * 512:(nh + 1) * 512], in0=ps,
                        scalar=ISW, in1=bvV_sb[:, nh * 512:(nh + 1) * 512],
                        op0=AluOp.mult, op1=AluOp.add)
        fr_x1T()

        # prefetch proj + GRU1 r-gate weights (SP queue, before attention)
        wprp = root.enter_context(tc.tile_pool(name="wprp", bufs=1, side="left"))
        wproj = wprp.tile([P, DC, D], FP8)
        nc.sync.dma_start(out=wproj, in_=wproj_d[:].rearrange("(kc p) n -> p kc n", p=P))
        wr1 = wprp.tile([P, DC, D], FP8)
        nc.sync.dma_start(out=wr1, in_=gw_d[(1, "Wr")][:].rearrange("(kc p) n -> p kc n", p=P))
        ur1 = wprp.tile([P, DC, D], FP8)
        nc.sync.dma_start(out=ur1, in_=gw_d[(1, "Ur")][:].rearrange("(kc p) n -> p kc n", p=P))

        # reserve GRU output tiles below the inp tiles (LIFO frees)
        o1T_f, fr_o1f = mk("o1T_f", (P, DC, CUR), F32, "left")
        o1_8, fr_o18 = mk("o1_8", (P, DC, CUR), FP8, "left")

        # load GRU1 inputs early (SP queue; needed in phase 4)
        inpT_f, fr_inpf = mk("inpT_f", (P, DC, CUR), F32, "left")
        inp_8, fr_inp8 = mk("inp_8", (P, DC, CUR), FP8, "left")
        nc.sync.dma_start(out=inpT_f, in_=inpT_d[:].rearrange("(kc p) t -> p kc t", p=P))
        for n in range(DC):
            VE(n).tensor_copy(inp_8[:, n, :], inpT_f[:, n, :])

        # ================= Phase 3: attention =================
        avT, fr_avT = mk("avT", (P, DC, CUR), FP8, "left")
        with ExitStack() as ph:
            scp = ph.enter_context(tc.tile_pool(name="scp", bufs=3, space="PSUM"))
            ptp = ph.enter_context(tc.tile_pool(name="ptp", bufs=1, space="PSUM"))
            avp = ph.enter_context(tc.tile_pool(name="avp", bufs=1, space="PSUM"))
            pbw = ph.enter_context(tc.tile_pool(name="pbw", bufs=2, side="left"))
            shw = ph.enter_context(tc.tile_pool(name="shw", bufs=9, side="left"))
            smw = ph.enter_context(tc.tile_pool(name="smw", bufs=3, side="left"))
            enw = ph.enter_context(tc.tile_pool(name="enw", bufs=3, side="left"))
            atw = ph.enter_context(tc.tile_pool(name="atw", bufs=2, side="left"))
            dnw = ph.enter_context(tc.tile_pool(name="dnw", bufs=3, side="left"))

            def hsl(t, h):
                ch, rb = h // 2, (h % 2) * HEAD_DIM
                return t[rb:rb + HEAD_DIM, ch, :]

            def head_pos(h):
                """Pos scores + shift round trip for head h."""
                qvh = hsl(qvT, h)
                rh = hsl(rT, h)
                s_t = scr[h % n_scr]
                pb = pbw.tile([P, TCC, 1024], FP8, name="pb")
                for ic in range(TCC - 1):
                    nc.gpsimd.memset(pb[:, ic, 0:384 - 128 * ic], 0.0)
                for ic in range(TCC):
                    c0 = 384 - 128 * ic          # first rel col needed
                    pp = scp.tile([P, 1024], F32, name="pp", tag="sc")
                    nc.tensor.matmul(pp[:, c0:512], lhsT=qvh[:, ic * P:(ic + 1) * P],
                                     rhs=rh[:, c0:512], start=True, stop=True)
                    nc.tensor.matmul(pp[:, 512:1024], lhsT=qvh[:, ic * P:(ic + 1) * P],
                                     rhs=rh[:, 512:1024], start=True, stop=True)
                    if ic % 2 == 0:
                        nc.scalar.copy(pb[:, ic, c0:1024], pp[:, c0:1024])
                    else:
                        nc.vector.tensor_copy(pb[:, ic, c0:1024], pp[:, c0:1024])
                # one combined scratch write for all 4 query chunks
                nc.sync.dma_start(out=s_t[:, :, 0:1024], in_=pb)
                shps = []
                for ic in range(TCC):
                    wr = (ic + 5) * 128          # shifted-read width
                    shp = shw.tile([P, FULL], FP8, name="shp")
                    shift_ap = bass.AP(tensor=s_t.tensor,
                                       offset=s_t.offset + 1536 * ic + 511 - 128 * ic,
                                       ap=[[TCC * 1536 - 1, P], [1, wr]])
                    nc.sync.dma_start(out=shp[:, 0:wr], in_=shift_ap)
                    shps.append(shp)
                return shps

            def head_content(h, shps):
                """Content scores + softmax numerators for head h."""
                quh = hsl(quT, h)
                kh = hsl(kT, h)
                esn = enw.tile([P, TCC, FULL], BF16, name="esn")
                den = dnw.tile([P, TCC], F32, name="den")
                rec = dnw.tile([P, TCC], F32, name="rec")
                for ic in range(TCC):
                    wr = (ic + 5) * 128
                    shp = shps[ic]
                    cp = scp.tile([P, 1024], F32, name="cp", tag="sc")
                    nc.tensor.matmul(cp[:, 0:512], lhsT=quh[:, ic * P:(ic + 1) * P],
                                     rhs=kh[:, 0:512], start=True, stop=False)
                    nc.tensor.matmul(cp[:, 512:wr], lhsT=quh[:, ic * P:(ic + 1) * P],
                                     rhs=kh[:, 512:wr], start=True, stop=False)
                    # add shifted pos scores in PSUM: cp += I^T @ shp
                    nc.tensor.matmul(cp[:, 0:512], lhsT=ident_8,
                                     rhs=shp[:, 0:512], start=False, stop=True)
                    nc.tensor.matmul(cp[:, 512:wr], lhsT=ident_8,
                                     rhs=shp[:, 512:wr], start=False, stop=True)
                    nc.scalar.activation(out=esn[:, ic, 0:wr], in_=cp[:, 0:wr],
                                         func=Act.Exp, scale=SCALE,
                                         accum_out=den[:, ic:ic + 1])
                    nc.vector.reciprocal(out=rec[:, ic:ic + 1],
                                         in_=den[:, ic:ic + 1])
                    nc.gpsimd.tensor_scalar(out=esn[:, ic, 0:wr],
                                            in0=esn[:, ic, 0:wr],
                                            scalar1=rec[:, ic:ic + 1],
                                            scalar2=ES_S, op0=AluOp.mult,
                                            op1=AluOp.mult)
                return esn, rec

            def head_tail(h, esn, rec):
                """Transpose + AV + normalize for head h (two heads behind)."""
                ch, rb = h // 2, (h % 2) * HEAD_DIM
                attnT = atw.tile([P, TCF, 512], FP8, name="attnT")
                nc.gpsimd.memset(attnT[:, 5, 0:128], 0.0)
                nc.gpsimd.memset(attnT[:, 7, 256:384], 0.0)
                for jc in range(TCF):
                    ic0 = max(0, jc - 4)
                    pt = ptp.tile([P, 512], BF16, name="pt", tag="pt")
                    for ic in range(ic0, TCC):
                        nc.tensor.transpose(pt[:, ic * P:(ic + 1) * P],
                                            esn[:, ic, jc * P:(jc + 1) * P],
                                            ident_b)
                    nc.vector.tensor_copy(attnT[:, jc, ic0 * P:512],
                                          pt[:, ic0 * P:512])
                av = avp.tile([P, 512], F32, name="av", tag="av")
                for qh in range(2):
                    pairs = [0, 1, 2] if qh == 0 else [0, 1, 2, 3]
                    for i, pr in enumerate(pairs):
                        nc.tensor.matmul(
                            av[0:HEAD_DIM, qh * 256:qh * 256 + 256],
                            lhsT=v_nat[:, 2 * pr:2 * pr + 2,
                                       h * HEAD_DIM:(h + 1) * HEAD_DIM],
                            rhs=attnT[:, 2 * pr:2 * pr + 2,
                                      qh * 256:qh * 256 + 256],
                            perf_mode=DR,
                            start=(i == 0), stop=(i == len(pairs) - 1))
                nc.vector.tensor_scalar_mul(avT[rb:rb + HEAD_DIM, ch, :],
                                            av[0:HEAD_DIM, :], IES)

            # depth-2 software pipeline: pos(h) | content(h-1) | tail(h-2)
            sh_st = {}
            cr_st = {}
            for it in range(HEAD_NUM + 2):
                if it < HEAD_NUM:
                    sh_st[it] = head_pos(it)
                if 1 <= it <= HEAD_NUM:
                    cr_st[it - 1] = head_content(it - 1, sh_st.pop(it - 1))
                if it >= 2:
                    esn, rec = cr_st.pop(it - 2)
                    head_tail(it - 2, esn, rec)
        fr_qvT(); fr_quT(); fr_rT(); fr_v(); fr_kT()

        # ================= Phase 4: proj + GRU1 =================
        psum_box["p"] = root.enter_context(
            tc.tile_pool(name="psum_d", bufs=4, space="PSUM"))
        psum_box["s"] = root.enter_context(
            tc.tile_pool(name="psum_sd", bufs=2, space="PSUM"))
        a1T, fr_a1T = mk("a1T", (P, DC, CUR), FP8, "right")
        for n in range(DC):
            ps = PS()
            _dr_quads(nc, ps, [(wproj, avT, n * P, 0, KP)])
            nc.scalar.activation(out=a1T[:, n, :], in_=ps, func=Act.Relu,
                                 scale=ISW, bias=bproj_sb[:, n:n + 1])
        fr_avT()

        with ExitStack() as ph:
            _gru(nc, tc, ph, PS, gw_d, 1, a1T, inp_8, inpT_f, nbg1_sb,
                 o1T_f, o1_8, VE, pre=(wr1, ur1))
        fr_inp8(); fr_inpf(); fr_a1T()

        # ================= Phase 5: LN2 =================
        x2T, fr_x2T = mk("x2T", (P, DC, CUR), FP8, "right")
        with ExitStack() as ph:
            lw = ph.enter_context(tc.tile_pool(name="lw", bufs=2, side="left"))
            sqp = ph.enter_context(tc.tile_pool(name="sqp", bufs=1, side="left"))
            sq = sqp.tile([P, DC, 512], FP8, name="sq")
            for n in range(DC):
                VE(n).tensor_mul(sq[:, n, :], o1_8[:, n, :], o1_8[:, n, :])
            s1 = SM()
            for n in range(DC):
                nc.tensor.matmul(s1, lhsT=ones_red8, rhs=o1_8[:, n, :],
                                 start=(n == 0), stop=(n == DC - 1))
            mean = lw.tile([1, 512], F32, name="mean")
            nc.vector.tensor_scalar_mul(mean, s1, 1.0 / D)
            s2 = SM()
            for n in range(DC):
                nc.tensor.matmul(s2, lhsT=ones_red8, rhs=sq[:, n, :],
                                 start=(n == 0), stop=(n == DC - 1))
            m2m = lw.tile([1, 512], F32, name="m2m")
            nc.vector.tensor_scalar_mul(m2m, s2, 1.0 / D)
            var = lw.tile([1, 512], F32, name="var")
            nc.vector.scalar_tensor_tensor(out=var, in0=mean, scalar=1.0,
                                           in1=mean, op0=AluOp.mult,
                                           op1=AluOp.mult)
            nc.vector.tensor_sub(var, m2m, var)
            sd = lw.tile([1, 512], F32, name="sd2")
            nc.scalar.activation(out=sd, in_=var, func=Act.Sqrt,
                                 bias=eps_t[0:1, :])
            rstd = lw.tile([1, 512], F32, name="rstd2")
            nc.vector.reciprocal(out=rstd, in_=sd)
            meanB = lw.tile([P, 512], F32, name="meanB")
            nc.gpsimd.partition_broadcast(meanB, mean)
            rstdB = lw.tile([P, 512], F32, name="rstdB")
            nc.gpsimd.partition_broadcast(rstdB, rstd)
            for n in range(DC):
                t1 = lw.tile([P, 512], F32, name="t1")
                VE(n).tensor_sub(t1, o1T_f[:, n, :], meanB)
                VE(n).tensor_mul(t1, t1, rstdB)
                VE(n + 1).tensor_scalar(out=x2T[:, n, :], in0=t1,
                                        scalar1=ln2g_sb[:, n:n + 1],
                                        scalar2=ln2b_sb[:, n:n + 1],
                                        op0=AluOp.mult, op1=AluOp.add)

        # ================= Phase 6: MLP =================
        with ExitStack() as ph6:
            m1w = ph6.enter_context(tc.tile_pool(name="m1w", bufs=1, side="right"))
            m1T = m1w.tile([P, HC, 512], FP8)
            w12p = ph6.enter_context(tc.tile_pool(name="w12p", bufs=1, side="right"))
            w1t = w12p.tile([P, DC, HID], FP8, name="w1t")
            nc.sync.dma_start(out=w1t, in_=w1_d[:].rearrange("(kc p) n -> p kc n", p=P))
            w2t = w12p.tile([P, HC, D], FP8, name="w2t")
            nc.sync.dma_start(out=w2t, in_=w2_d[:].rearrange("(kc p) n -> p kc n", p=P))
            for n in range(HC):
                ps = PS()
                _dr_quads(nc, ps, [(w1t, x2T, n * P, 0, KP)])
                nc.scalar.activation(out=m1T[:, n, :], in_=ps, func=Act.Relu,
                                     scale=ISW, bias=b1_sb[:, n:n + 1])
            m2T, fr_m2T = mk("m2T", (P, DC, CUR), FP8, "left")
            KPH = list(range(HC // 2))
            for n in range(DC):
                ps = PS()
                _dr_quads(nc, ps, [(w2t, m1T, n * P, 0, KPH)])
                nc.scalar.activation(out=m2T[:, n, :], in_=ps, func=Act.Relu,
                                     scale=ISW, bias=b2_sb[:, n:n + 1])
        fr_x2T()

        # ================= Phase 7: GRU2 + output =================
        o2T_f, fr_o2 = mk("o2T_f", (P, DC, CUR), F32, "right")
        o2r = out_d[:].rearrange("(kc p) t -> p kc t", p=P)

        def out_chunk(n):
            nc.sync.dma_start(out=o2r[:, n, :], in_=o2T_f[:, n, :])

        with ExitStack() as ph:
            _gru(nc, tc, ph, PS, gw_d, 2, m2T, o1_8, o1T_f, nbg2_sb,
                 o2T_f, None, VE, on_chunk=out_chunk)
        fr_m2T(); fr_o18(); fr_o1f()
        fr_o2()


def _gru(nc, tc, ph, PS, gw_d, g, yT, x8, xf, nbg_sb, oT_f, o_8, VE,
         pre=None, on_chunk=None):
    gwp = ph.enter_context(tc.tile_pool(name=f"gw{g}", bufs=4, side="left"))
    gtmp = ph.enter_context(tc.tile_pool(name=f"gt{g}", bufs=3, side="left"))
    gper = ph.enter_context(tc.tile_pool(name=f"gp{g}", bufs=1, side="left"))
    KP = [0, 1, 2, 3]

    def loadw(m):
        w = gwp.tile([P, DC, D], FP8, name=f"gwt_{m}", tag="gwt")
        nc.sync.dma_start(out=w, in_=gw_d[(g, m)][:].rearrange("(kc p) n -> p kc n", p=P))
        return w

    wr, ur = pre if pre is not None else (loadw("Wr"), loadw("Ur"))
    wz, uz = loadw("Wz"), loadw("Uz")
    rx = gper.tile([P, DC, 512], FP8, name="rx")
    for n in range(DC):
        ps = PS()
        _dr_quads(nc, ps, [(wr, yT, n * P, 0, KP), (ur, x8, n * P, 0, KP)])
        rr = gtmp.tile([P, 512], F32, name="rr")
        nc.scalar.activation(out=rr, in_=ps, func=Act.Sigmoid, scale=ISW)
        VE(n).tensor_mul(rx[:, n, :], rr, xf[:, n, :])
    wg, ug = loadw("Wg"), loadw("Ug")
    zt = gper.tile([P, DC, 512], BF16, name="zt")
    for n in range(DC):
        ps = PS()
        _dr_quads(nc, ps, [(wz, yT, n * P, 0, KP), (uz, x8, n * P, 0, KP)])
        nc.scalar.activation(out=zt[:, n, :], in_=ps, func=Act.Sigmoid,
                             scale=ISW, bias=nbg_sb[:, n:n + 1])
    for n in range(DC):
        ps = PS()
        _dr_quads(nc, ps, [(wg, yT, n * P, 0, KP), (ug, rx, n * P, 0, KP)])
        ht = gtmp.tile([P, 512], F32, name="ht")
        nc.scalar.activation(out=ht, in_=ps, func=Act.Tanh, scale=ISW)
        VE(n).tensor_sub(ht, ht, xf[:, n, :])
        VE(n + 1).tensor_mul(ht, ht, zt[:, n, :])
        VE(n).tensor_add(oT_f[:, n, :], ht, xf[:, n, :])
        if o_8 is not None:
            VE(n + 1).tensor_copy(o_8[:, n, :], oT_f[:, n, :])
        if on_chunk is not None:
            on_chunk(n)


_NC_CACHE = {}


def _get_nc():
    if "nc" not in _NC_CACHE:
        _NC_CACHE["nc"] = _build()
    return _NC_CACHE["nc"]


def _chunk_t(vec):
    n = vec.shape[0] // P
    return np.ascontiguousarray(vec.reshape(n, P).T.astype(np.float32))


def _fp8w(w):
    f8 = ml_dtypes.float8_e4m3
    return np.clip(np.asarray(w, np.float32) * WS, -240.0, 240.0).astype(f8)


def _prep(inputs):
    f32 = np.float32
    f8 = ml_dtypes.float8_e4m3
    inp = np.asarray(inputs["inputs"], f32)
    mem = np.asarray(inputs["memory"], f32)
    pos = np.asarray(inputs["pos_embedding"], f32)[:, 0, :]
    wkv = np.asarray(inputs["Wkv"], f32)

    shared = {
        "posT8": np.clip(np.ascontiguousarray(pos.T), -240, 240).astype(f8),
        "u_t": _chunk_t(np.asarray(inputs["u"], f32).reshape(-1)),
        "v_t": _chunk_t(np.asarray(inputs["v"], f32).reshape(-1)),
        "ln1_g_t": _chunk_t(np.asarray(inputs["ln1_g"], f32)),
        "ln1_b_t": _chunk_t(np.asarray(inputs["ln1_b"], f32)),
        "ln2_g_t": _chunk_t(np.asarray(inputs["ln2_g"], f32)),
        "ln2_b_t": _chunk_t(np.asarray(inputs["ln2_b"], f32)),
        "bkvK_t": _chunk_t(np.asarray(inputs["bkv"], f32)[0:D]),
        "bkvV_row": np.asarray(inputs["bkv"], f32)[D:2 * D].reshape(1, D),
        "bq_t": _chunk_t(np.asarray(inputs["bq"], f32)),
        "bpos_t": _chunk_t(np.asarray(inputs["bpos"], f32)),
        "bproj_t": _chunk_t(np.asarray(inputs["bproj"], f32)),
        "b1_t": _chunk_t(np.asarray(inputs["mlp_b1"], f32)),
        "b2_t": _chunk_t(np.asarray(inputs["mlp_b2"], f32)),
        "nbg1_t": _chunk_t(-np.asarray(inputs["g1_bg"], f32)),
        "nbg2_t": _chunk_t(-np.asarray(inputs["g2_bg"], f32)),
        "WkvK8": _fp8w(wkv[:, 0:D]),
        "WkvV8": _fp8w(wkv[:, D:2 * D]),
        "Wq8": _fp8w(inputs["Wq"]),
        "Wpos8": _fp8w(inputs["Wpos"]),
        "Wproj8": _fp8w(inputs["Wproj"]),
        "mlp_W18": _fp8w(inputs["mlp_W1"]),
        "mlp_W28": _fp8w(inputs["mlp_W2"]),
    }
    for g in (1, 2):
        for m in ("Wr", "Ur", "Wz", "Uz", "Wg", "Ug"):
            shared[f"g{g}_{m}8"] = _fp8w(inputs[f"g{g}_{m}"])

    in_maps = []
    for b in range(BS):
        im = dict(shared)
        im["x_full"] = np.ascontiguousarray(
            np.concatenate([mem[:, b, :], inp[:, b, :]], axis=0).T)
        im["inpT"] = np.ascontiguousarray(inp[:, b, :].T)
        in_maps.append(im)
    return in_maps


def _post(out_t):
    """Device output is [D, CUR]; transpose to [CUR, D]."""
    return np.ascontiguousarray(np.asarray(out_t).T.astype(np.float32))


def kernel(**inputs):
    nc = _get_nc()
    in_maps = _prep(inputs)
    res = run_bass_kernel_spmd(nc, in_maps, core_ids=list(range(BS)))
    out = np.stack([_post(res.results[b]["out"]) for b in range(BS)], axis=1)
    return np.ascontiguousarray(out.astype(np.float32))


if __name__ == "__main__":
    _get_nc()
    print("build+compile OK")
